# revision 4
# baseline (speedup 1.0000x reference)
"""Binarized 3x3 conv (BConv) Trainium2 Bass kernel.

Problem: x[32,256,56,56] f32, W[256,256,3,3] f32.
  out = conv2d(x, sign(W), stride 1, pad 1)  (NCHW / OIHW)

Strategy:
  - Data-parallel over batch: 8 cores x 4 images each, identical SPMD program.
  - Per core: conv as 9 shifted matmuls (one per kernel tap) x 2 input-channel
    halves, accumulated in PSUM. Weights binarized on-device (PE transpose ->
    ACT Sign -> bf16 [C_in,C_out] tiles). Activations cast f32->bf16 into a
    zero-padded [128,58,58] SBUF image so no edge masking is needed.
  - Output tiles [128 out-ch, 8 rows, 56 cols] (N=448 <= one PSUM bank).
    7 row-tiles per image share one weight load sweep (18 taps x 7 tiles).
"""

import sys
from contextlib import ExitStack

sys.path.insert(0, "/opt/trn_rl_repo")

import numpy as np

import concourse.bass as bass
import concourse.mybir as mybir
import concourse.tile as tile
from concourse import bacc
from concourse.bass_utils import run_bass_kernel_spmd
from concourse.masks import make_identity

N_CORES = 8
NIMG = 4          # images per core (32 / 8)
C = 256           # channels (in == out)
H = 56
HP = H + 2        # padded spatial
P = 128           # partitions
ROWS_PER_TILE = 8         # output rows per PSUM tile -> N = 8*56 = 448
NFT = H // ROWS_PER_TILE  # 7 row-tiles per image

F32 = mybir.dt.float32
BF16 = mybir.dt.bfloat16

_cached = {}


def build_program():
    nc = bacc.Bacc("TRN2", target_bir_lowering=False, debug=False,
                   num_devices=N_CORES)

    x_d = nc.dram_tensor("x", [NIMG, C, H, H], F32, kind="ExternalInput")
    w_d = nc.dram_tensor("W", [C, C * 9], F32, kind="ExternalInput")
    y_d = nc.dram_tensor("y", [NIMG, C, H, H], F32, kind="ExternalOutput")

    with tile.TileContext(nc) as tc, ExitStack() as ctx:
        const_pool = ctx.enter_context(tc.tile_pool(name="const", bufs=1))
        wstage_pool = ctx.enter_context(tc.tile_pool(name="wstage", bufs=2))
        wt_pool = ctx.enter_context(tc.tile_pool(name="wt", bufs=36))
        pad_pool = ctx.enter_context(tc.tile_pool(name="pad", bufs=4))
        stage_pool = ctx.enter_context(tc.tile_pool(name="stage", bufs=3))
        out_pool = ctx.enter_context(tc.tile_pool(name="osb", bufs=6))
        psum_pool = ctx.enter_context(tc.tile_pool(name="ps", bufs=8,
                                                   space="PSUM"))

        ident = const_pool.tile([P, P], F32)
        make_identity(nc, ident[:])

        # ---- weights: DMA [128,2304] f32, PE-transpose each 128x128 tap
        # slice to [C_in, C_out], binarize via ACT Sign -> bf16.
        wt = {}  # (tap, ic, oc) -> [128,128] bf16 tile with wt[i, o]
        for oc in range(2):
            wst = wstage_pool.tile([P, 2 * P, 9], F32, tag="wst")
            nc.sync.dma_start(wst[:, :, :], w_d[oc * P:(oc + 1) * P, :])
            for ic in range(2):
                for k in range(9):
                    ps = psum_pool.tile([P, P], F32, tag="ps")
                    nc.tensor.transpose(
                        ps[:], wst[:, ic * P:(ic + 1) * P, k], ident[:])
                    t = wt_pool.tile([P, P], BF16, tag="wt")
                    nc.scalar.sign(t[:], ps[:])
                    wt[(k, ic, oc)] = t

        # ---- main loop over images
        for img in range(NIMG):
            pads = []
            for ic in range(2):
                pad = pad_pool.tile([P, HP, HP], BF16, tag="pad")
                # zero only the 1-px border; interior fully overwritten
                nc.gpsimd.memset(pad[:, 0, :], 0.0)
                nc.gpsimd.memset(pad[:, HP - 1, :], 0.0)
                nc.gpsimd.memset(pad[:, 1:HP - 1, 0], 0.0)
                nc.gpsimd.memset(pad[:, 1:HP - 1, HP - 1], 0.0)
                stg = stage_pool.tile([P, H, H], F32, tag="stage")
                nc.sync.dma_start(stg[:, :, :], x_d[img, ic * P:(ic + 1) * P])
                # f32 -> bf16 cast into padded interior (ScalarE)
                nc.scalar.copy(pad[:, 1:HP - 1, 1:HP - 1], stg[:, :, :])
                pads.append(pad)

            for oc in range(2):
                psums = [psum_pool.tile([P, ROWS_PER_TILE, H], F32, tag="ps",
                                        name=f"acc_{img}_{oc}_{f}")
                         for f in range(NFT)]
                step = 0
                for k in range(9):
                    dh, dw = k // 3, k % 3
                    for ic in range(2):
                        w_tile = wt[(k, ic, oc)]
                        for f in range(NFT):
                            r0 = f * ROWS_PER_TILE + dh
                            nc.tensor.matmul(
                                psums[f][:],
                                w_tile[:],
                                pads[ic][:, r0:r0 + ROWS_PER_TILE, dw:dw + H],
                                start=(step == 0),
                                stop=(step == 17),
                            )
                        step += 1
                for f in range(NFT):
                    osb = out_pool.tile([P, ROWS_PER_TILE, H], F32, tag="osb")
                    nc.vector.tensor_copy(osb[:], psums[f][:])
                    nc.sync.dma_start(
                        y_d[img, oc * P:(oc + 1) * P,
                            f * ROWS_PER_TILE:(f + 1) * ROWS_PER_TILE, :],
                        osb[:],
                    )

    nc.compile()
    return nc


def _get_program():
    if "nc" not in _cached:
        _cached["nc"] = build_program()
    return _cached["nc"]


def kernel(x: np.ndarray, W: np.ndarray, trace: bool = False, **trace_kw):
    nc = _get_program()
    x = np.ascontiguousarray(x, dtype=np.float32)
    w_flat = np.ascontiguousarray(W.reshape(C, C * 9), dtype=np.float32)
    in_maps = [{"x": x[i * NIMG:(i + 1) * NIMG], "W": w_flat}
               for i in range(N_CORES)]
    res = run_bass_kernel_spmd(nc, in_maps, core_ids=list(range(N_CORES)),
                               trace=trace, **trace_kw)
    out = np.concatenate([res.results[i]["y"] for i in range(N_CORES)], axis=0)
    if trace:
        return out, res
    return out
